# revision 15
# baseline (speedup 1.0000x reference)
"""Trainium2 Bass kernel for nn_ARMA (gnn_message_passing), 8 NeuronCores.

Math (reference refactored):
    Lap(x)[t] = mask_t * x[t] - (1/deg_t) * sum_{e: tgt_e = t} x[src_e]
    (biases inside the conv vanish through Lap exactly)
    out1 = relu(Lap(data) @ W_pre + data @ T_pre + bT_pre)
    out2 = relu(gl(Lap(out1), W_blk) + data @ T_blk + bT_blk)
    out3 = relu(gl(Lap(out2), W_post) + data @ T_post + bT_post)
    final[n, i] = mean_j out3[n, 3i + j]
where gl() is a per-node grouped linear == dense block-diagonal matmul and
Lap commutes with per-node linear maps (so it is applied before them).

Sharding: contiguous target-node ranges (49 blocks of 128 nodes per core).
Each core holds the full gather source (data / allgathered layer outputs)
in HBM; the per-edge gather + segmented-sum runs only over the core's own
~100k edges. Inter-layer halo exchange = AllGather of each core's
[6272, 128] bf16 output slab.

Per-block segmented sum: the per-edge gather lands edge rows in SBUF
partitions (spread over the 4 SWDGE queues); a chunked indicator matmul
accumulates them per target in PSUM.  All 18 indicator chunks for a
block are built up front into one wide SBUF tile with 4-chunk-wide
is_equal ops (vector-engine streak), then the 18 matmuls run as one PE
streak (the first resets PSUM via start=True).  Versus an
is_equal/matmul ping-pong this cuts engine handoffs per block from ~38
to ~7, which this runtime charges heavily for (~30% end-to-end).  The
lo/hi gather split sits at LO=25088 (not 32768) so both int16 index
regions see ~equal edge counts, shrinking the worst-case chunk count
per block from 19 to 18 and balancing the four SWDGE queues.  One
identity matmul transposes the node-major sum back to feature-major.
"""

import os

os.environ.setdefault("JAX_COMPILATION_CACHE_DIR", "/tmp/jax_neff_cache")

import numpy as np
import ml_dtypes

import concourse.bacc as bacc
import concourse.bass as bass
import concourse.mybir as mybir
import concourse.tile as tile
from concourse.bass_utils import run_bass_kernel_spmd

BF16 = mybir.dt.bfloat16
F32 = mybir.dt.float32
I16 = mybir.dt.int16
DS = bass.ds

# -------------------- problem constants --------------------
N = 50000
IN = 128
H = 32
WIDTH = 3
HW = H * WIDTH    # 96
OUT = 64
OW = OUT * WIDTH  # 192
NCORES = 8
BLK = 128
NBC = ((N + NCORES - 1) // NCORES + BLK - 1) // BLK  # 49 blocks/core
NPC = NBC * BLK                                      # 6272
NPAD = NPC * NCORES                                  # 50176
LO = 25088


# -------------------- host preprocessing --------------------
def preprocess(src, tgt):
    """Per-core gather index / indicator-target / degree arrays.

    idx_lo [128, NBC*CL*8] int16 (16-wrapped, replicated x8 over partitions)
    idx_hi [128, NBC*CH*8] int16
    tl     [128, NBC*CPBE] bf16 (slot -> target-local id, -1 padding;
                                 per-block: CL lo chunks then CH hi chunks,
                                 then a possible all-pad chunk so the chunk
                                 count per block is even)
    """
    src = np.asarray(src).astype(np.int64)
    tgt = np.asarray(tgt).astype(np.int64)
    E = src.shape[0]
    if np.any(np.diff(tgt) < 0):
        order = np.argsort(tgt, kind="stable")
        src, tgt = src[order], tgt[order]

    deg = np.bincount(tgt, minlength=NPAD).astype(np.float64)
    a = (deg > 0).astype(np.float32)
    b = np.where(deg > 0, 1.0 / np.maximum(deg, 1), 0.0).astype(np.float32)
    all_deg_pos = bool(deg[:N].min() >= 1)

    blk = tgt >> 7
    region = (src >= LO).astype(np.int64)
    key = blk * 2 + region
    order = np.argsort(key, kind="stable")
    src_s, key_s = src[order], key[order]
    tgt_s = tgt[order]

    nblk = NBC * NCORES
    cnt = np.bincount(key, minlength=nblk * 2).reshape(nblk, 2)
    CL = max(1, int(-(-cnt[:, 0].max() // BLK)))
    CH = max(1, int(-(-cnt[:, 1].max() // BLK))) if NPAD > LO else 0
    CPB = CL + CH
    CPBE = -(-CPB // 2) * 2  # chunk count, mult of 2

    starts = np.zeros(nblk * 2, dtype=np.int64)
    np.cumsum(cnt.reshape(-1)[:-1], out=starts[1:])
    pos = np.arange(E, dtype=np.int64) - starts[key_s]

    core = blk[order] // NBC
    b_loc = blk[order] % NBC

    is_lo = key_s % 2 == 0
    slot_lo = b_loc * (CL * BLK) + pos
    slot_hi = b_loc * (CH * BLK) + pos

    idx_lo = np.zeros((NCORES, NBC * CL * BLK), dtype=np.int16)
    idx_hi = np.zeros((NCORES, max(NBC * CH * BLK, 1)), dtype=np.int16)
    tl = np.full((NCORES, NBC * CPBE * BLK), -1.0, dtype=np.float32)

    li = is_lo
    hm = ~is_lo
    idx_lo[core[li], slot_lo[li]] = src_s[li].astype(np.int16)
    if CH > 0:
        idx_hi[core[hm], slot_hi[hm]] = (src_s[hm] - LO).astype(np.int16)

    tloc = (tgt_s & 127).astype(np.float32)
    tsl_lo = b_loc * (CPBE * BLK) + (pos // BLK) * BLK + (pos % BLK)
    tsl_hi = b_loc * (CPBE * BLK) + (CL + pos // BLK) * BLK + (pos % BLK)
    tl[core[li], tsl_lo[li]] = tloc[li]
    if CH > 0:
        tl[core[hm], tsl_hi[hm]] = tloc[hm]

    out = {"CL": CL, "CH": CH, "all_deg_pos": all_deg_pos, "cores": []}
    for c in range(NCORES):
        wl = np.tile(idx_lo[c].reshape(-1, 16).T, (8, 1)).copy()
        if CH > 0:
            wh = np.tile(idx_hi[c].reshape(-1, 16).T, (8, 1)).copy()
        else:
            wh = np.zeros((128, 8), dtype=np.int16)
        tlc = tl[c].reshape(-1, BLK).T.astype(ml_dtypes.bfloat16).copy()
        sl = slice(c * NPC, (c + 1) * NPC)
        out["cores"].append({
            "idx_lo": wl, "idx_hi": wh, "tl": tlc,
            "a_own": a[sl][None, :].copy(),
            "b_own": b[sl][None, :].copy(),
        })
    return out


# weight blob column layout (bf16, [128, 768])
#   w1 [128,96] @0, w2 [96,96] @96, w3 [96,192] @192,
#   t1 [128,96] @384, t2 [128,96] @480, t3 [128,192] @576
# const blob (bf16, [128, 768])
#   iota [128,128] @0, mm1 [128,64] @128, mm2 [64,64] @192,
#   bt1 [1,96] @256 (row 0), bt2 [1,96] @352, bt3 [1,192] @448,
#   identity [128,128] @640
WBLOB_COLS = 768
CBLOB_COLS = 768


def prep_weights(W_pre, T_pre, bT_pre, W_blk, T_blk, bT_blk,
                 W_post, T_post, bT_post):
    bf = ml_dtypes.bfloat16
    wb = np.zeros((128, WBLOB_COLS), np.float32)
    wb[:, 0:96] = np.asarray(W_pre, np.float32)
    W_blk = np.asarray(W_blk, np.float32)
    W_post = np.asarray(W_post, np.float32)
    for g in range(WIDTH):
        wb[g * H:(g + 1) * H, 96 + g * H:96 + (g + 1) * H] = W_blk[g]
        wb[g * H:(g + 1) * H, 192 + g * OUT:192 + (g + 1) * OUT] = W_post[g]
    wb[:, 384:480] = np.asarray(T_pre, np.float32)
    wb[:, 480:576] = np.asarray(T_blk, np.float32)
    wb[:, 576:768] = np.asarray(T_post, np.float32)

    cb = np.zeros((128, CBLOB_COLS), np.float32)
    cb[:, 0:128] = np.arange(128, dtype=np.float32)[None, :]
    m = np.zeros((OW, OUT), np.float32)
    for i in range(OUT):
        m[3 * i:3 * i + 3, i] = 1.0
    cb[:, 128:192] = m[:128]
    cb[:64, 192:256] = m[128:]
    cb[0, 256:352] = np.asarray(bT_pre, np.float32)
    cb[0, 352:448] = np.asarray(bT_blk, np.float32)
    cb[0, 448:640] = np.asarray(bT_post, np.float32)
    cb[:, 640:768] = np.eye(128, dtype=np.float32)

    has_bias = bool(
        np.any(np.asarray(bT_pre)) or np.any(np.asarray(bT_blk))
        or np.any(np.asarray(bT_post))
    )
    return wb.astype(bf), cb.astype(bf), has_bias


# -------------------- device program --------------------
def build_nc(CL, CH, has_bias, all_deg_pos=False, reps=1):
    CPB = CL + CH
    CPBE = -(-CPB // 2) * 2
    nc = bacc.Bacc("TRN2", target_bir_lowering=False, debug=False,
                   num_devices=NCORES, num_swdge_queues=4)

    data_sh = nc.declare_dram_parameter("data_sh", [NPC, IN], BF16, isOutput=False)
    idx_lo = nc.declare_dram_parameter("idx_lo", [16, NBC * CL * 8], I16, isOutput=False)
    idx_hi = nc.declare_dram_parameter("idx_hi", [16, max(NBC * CH * 8, 8)], I16, isOutput=False)
    tlp = nc.declare_dram_parameter("tl", [128, NBC * CPBE], BF16, isOutput=False)
    a_own = nc.declare_dram_parameter("a_own", [1, NPC], BF16, isOutput=False)
    b_own = nc.declare_dram_parameter("b_own", [1, NPC], F32, isOutput=False)
    wblob = nc.declare_dram_parameter("wblob", [128, WBLOB_COLS], BF16, isOutput=False)
    cblob = nc.declare_dram_parameter("cblob", [128, CBLOB_COLS], BF16, isOutput=False)
    out_ext = nc.declare_dram_parameter("out", [NPC, OUT], F32, isOutput=True)

    rg = [list(range(NCORES))]

    # 4-queue split of the per-block gather chunk list
    h1 = (CL + 1) // 2
    h2 = (CH + 1) // 2
    qsplit = [(0, h1, True), (h1, CL - h1, True),
              (CL, h2, False), (CL + h2, CH - h2, False)]
    qsplit = [(q, c0, nch, lo) for q, (c0, nch, lo) in enumerate(qsplit)
              if nch > 0]

    with tile.TileContext(nc) as tc:
        with (
            tc.tile_pool(name="res", bufs=1) as res,
            tc.tile_pool(name="lp", bufs=1) as lp,
            tc.tile_pool(name="xtp", bufs=2) as xtp,
            tc.tile_pool(name="pm", bufs=2, space="PSUM") as pmp,
            tc.tile_pool(name="po", bufs=2, space="PSUM") as pop,
            tc.tile_pool(name="pf", bufs=2, space="PSUM") as pfp,
            tc.tile_pool(name="dram", bufs=1, space="DRAM") as dram,
        ):
            # ---- residents
            data_stage = dram.tile([NPC, IN], BF16, name="data_stage")
            data_bf = dram.tile([NPAD, IN], BF16, addr_space="Shared",
                                name="data_full")
            nc.sync.dma_start(data_stage[:], data_sh[:, :])
            nc.gpsimd.collective_compute(
                "AllGather", mybir.AluOpType.bypass,
                replica_groups=[list(range(NCORES))],
                ins=[data_stage.opt()], outs=[data_bf.opt()],
            )
            dataT = res.tile([128, NPC], BF16)
            nc.sync.dma_start(out=dataT[:], in_=data_sh[:, :], transpose=True)
            a_rep = res.tile([128, NPC], BF16)
            nc.gpsimd.dma_start(out=a_rep[:], in_=a_own[:, :].broadcast_to([128, NPC]))
            b_rep = res.tile([128, NPC], F32)
            nc.gpsimd.dma_start(out=b_rep[:], in_=b_own[:, :].broadcast_to([128, NPC]))
            idxl = res.tile([128, NBC * CL * 8], I16)
            idxh = res.tile([128, max(NBC * CH * 8, 8)], I16)
            for k in range(8):
                nc.sync.dma_start(idxl[16 * k:16 * (k + 1), :], idx_lo[:, :])
                nc.sync.dma_start(idxh[16 * k:16 * (k + 1), :], idx_hi[:, :])
            tlt = res.tile([128, NBC * CPBE], BF16)
            nc.sync.dma_start(tlt[:], tlp[:, :])
            wb = res.tile([128, WBLOB_COLS], BF16)
            nc.sync.dma_start(wb[:], wblob[:, :])
            cb = res.tile([128, CBLOB_COLS], BF16)
            nc.sync.dma_start(cb[:], cblob[:, :])
            ones1 = res.tile([1, 128], BF16)
            nc.vector.memset(ones1[:], 1.0)
            ob = res.tile([128, IN], BF16)
            nc.vector.memset(ob[:], 0.0)
            IA = res.tile([128, CPBE * BLK], BF16)
            iota4 = res.tile([128, 4 * BLK], BF16)
            # gather staging: CPBE chunks; the pad chunk (if any) must hold
            # finite values since a zero indicator still multiplies it.
            g = res.tile([128, CPBE * BLK], BF16)
            if CPBE != CPB:
                nc.vector.memset(g[:, CPB * BLK:], 0.0)

            # blob views
            Wl_ = [wb[0:128, 0:96], wb[0:96, 96:192], wb[0:96, 192:384]]
            Tl_ = [wb[0:128, 384:480], wb[0:128, 480:576], wb[0:128, 576:768]]
            iota = cb[0:128, 0:128]
            for k in range(4):
                nc.vector.tensor_copy(iota4[:, DS(k * 128, 128)], iota)
            mm1 = cb[0:128, 128:192]
            mm2 = cb[0:64, 192:256]
            bT_ = [cb[0:1, 256:352], cb[0:1, 352:448], cb[0:1, 448:640]]
            idm = cb[0:128, 640:768]

            for _rep in range(reps):
                bounce = [
                    dram.tile([NPC, IN], BF16, name=f"bounce{l}_{_rep}",
                              tag=f"bounce{l}")
                    for l in (1, 2)
                ]
                yfull = [
                    dram.tile([NPAD, IN], BF16, addr_space="Shared",
                              name=f"yfull{l}_{_rep}")
                    for l in (1, 2)
                ]
                xT_cur = dataT

                for l in (1, 2, 3):
                    src_dram = data_bf if l == 1 else yfull[l - 2]
                    Kin = IN if l == 1 else HW
                    Wt, Tt, bTt = Wl_[l - 1], Tl_[l - 1], bT_[l - 1]

                    with tc.For_i(0, NBC, 1) as i:
                        for qn, c0, nch, lo_part in qsplit:
                            idxs = idxl if lo_part else idxh
                            ibase = i * (CL * 8) if lo_part else i * (CH * 8)
                            ioff = c0 * 8 if lo_part else (c0 - CL) * 8
                            nc.gpsimd.dma_gather(
                                out_ap=g[:, c0 * BLK:(c0 + nch) * BLK].rearrange(
                                    "p (c e) -> p c e", e=IN),
                                in_ap=(src_dram[0:LO, :] if lo_part
                                       else src_dram[LO:NPAD, :]),
                                idxs_ap=idxs[:, DS(ibase + ioff, nch * 8)],
                                num_idxs=nch * BLK, num_idxs_reg=nch * BLK,
                                elem_size=IN, single_packet=False,
                                queue_num=qn,
                            )
                        # node-major segmented sum: pmT[t, f]
                        pmT = pmp.tile([128, 128], F32, tag="pm")
                        _off = 0
                        while _off < CPBE:
                            _w = min(4, CPBE - _off)
                            nc.vector.tensor_tensor(
                                IA[:, DS(_off * BLK, _w * BLK)].rearrange(
                                    "p (c e) -> p c e", e=BLK),
                                tlt[:, DS(i * CPBE + _off, _w)].to_broadcast(
                                    [128, _w, BLK]),
                                iota4[:, :_w * BLK].rearrange(
                                    "p (c e) -> p c e", e=BLK),
                                op=mybir.AluOpType.is_equal,
                            )
                            _off += _w
                        for c in range(CPBE):
                            nc.tensor.matmul(
                                pmT[:], lhsT=IA[:, DS(c * BLK, BLK)],
                                rhs=g[:, DS(c * BLK, BLK)],
                                start=(c == 0), stop=False,
                            )
                        # transpose back to feature-major: pm [f, t]
                        cpy = lp.tile([128, 128], BF16, tag="cpy")
                        nc.scalar.activation(
                            cpy[:], pmT[:], mybir.ActivationFunctionType.Copy)
                        pm = pop.tile([128, 128], F32, tag="po")
                        nc.tensor.matmul(pm[:], lhsT=cpy[:], rhs=idm,
                                         start=True, stop=True)

                        dtb = lp.tile([128, 128], BF16, tag="dtb")
                        nc.vector.tensor_copy(dtb[:], dataT[:, DS(i * 128, 128)])
                        tB = lp.tile([128, 128], F32, tag="tB")
                        nc.vector.tensor_mul(tB[:], pm[:], b_rep[:, DS(i * 128, 128)])
                        lap = lp.tile([128, 128], BF16, tag="lap")
                        if all_deg_pos:
                            nc.vector.tensor_sub(
                                lap[:], xT_cur[:, DS(i * 128, 128)], tB[:])
                        else:
                            tA = lp.tile([128, 128], F32, tag="tA")
                            nc.vector.tensor_mul(
                                tA[:], xT_cur[:, DS(i * 128, 128)],
                                a_rep[:, DS(i * 128, 128)])
                            nc.vector.tensor_sub(lap[:], tA[:], tB[:])

                        if l < 3:
                            po = pmp.tile([128, 128], F32, tag="pm")
                            nc.tensor.matmul(po[:, :HW], lhsT=lap[:Kin, :], rhs=Wt,
                                             start=True, stop=False)
                            nc.tensor.matmul(po[:, :HW], lhsT=dtb[:], rhs=Tt,
                                             start=False, stop=not has_bias)
                            if has_bias:
                                nc.tensor.matmul(po[:, :HW], lhsT=ones1[:], rhs=bTt,
                                                 start=False, stop=True)
                            nc.scalar.activation(
                                ob[:, :HW], po[:, :HW],
                                mybir.ActivationFunctionType.Relu)
                            nc.sync.dma_start(
                                bounce[l - 1][DS(i * 128, 128), :], ob[:])
                        else:
                            pa = pmp.tile([128, 128], F32, tag="pm")
                            nc.tensor.matmul(pa[:], lhsT=Wt[:, 0:128],
                                             rhs=lap[:Kin, :], start=True, stop=False)
                            nc.tensor.matmul(pa[:], lhsT=Tt[:, 0:128], rhs=dtb[:],
                                             start=False, stop=not has_bias)
                            if has_bias:
                                nc.tensor.matmul(pa[:], lhsT=bTt[:, 0:128],
                                                 rhs=ones1[:], start=False, stop=True)
                            pb = pop.tile([128, 128], F32, tag="po")
                            nc.tensor.matmul(pb[:OW - 128, :], lhsT=Wt[:, 128:OW],
                                             rhs=lap[:Kin, :], start=True, stop=False)
                            nc.tensor.matmul(pb[:OW - 128, :], lhsT=Tt[:, 128:OW],
                                             rhs=dtb[:], start=False,
                                             stop=not has_bias)
                            if has_bias:
                                nc.tensor.matmul(pb[:OW - 128, :],
                                                 lhsT=bTt[:, 128:OW], rhs=ones1[:],
                                                 start=False, stop=True)
                            o3a = lp.tile([128, 128], BF16, tag="tA")
                            nc.scalar.activation(
                                o3a[:], pa[:], mybir.ActivationFunctionType.Relu)
                            o3b = lp.tile([128, 128], BF16, tag="tB2")
                            nc.scalar.activation(
                                o3b[:OW - 128, :], pb[:OW - 128, :],
                                mybir.ActivationFunctionType.Relu)
                            pf = pfp.tile([128, OUT], F32, tag="pf")
                            nc.tensor.matmul(pf[:], lhsT=o3a[:], rhs=mm1,
                                             start=True, stop=False)
                            nc.tensor.matmul(pf[:], lhsT=o3b[:OW - 128, :], rhs=mm2,
                                             start=False, stop=True)
                            of = lp.tile([128, OUT], F32, tag="of")
                            nc.scalar.activation(
                                of[:], pf[:], mybir.ActivationFunctionType.Copy,
                                scale=1.0 / 3.0)
                            nc.sync.dma_start(out_ext[DS(i * 128, 128), :], of[:])

                    if l < 3:
                        nc.gpsimd.collective_compute(
                            "AllGather", mybir.AluOpType.bypass,
                            replica_groups=rg,
                            ins=[bounce[l - 1].opt()],
                            outs=[yfull[l - 1].opt()],
                        )
                        xT_new = xtp.tile([128, NPC], BF16, tag="xT")
                        nc.sync.dma_start(out=xT_new[:], in_=bounce[l - 1][:, :],
                                          transpose=True)
                        xT_cur = xT_new

    nc.compile()
    return nc


# -------------------- kernel entry --------------------
_CACHE = {}


def _get_nc(CL, CH, has_bias, all_deg_pos):
    key = (CL, CH, has_bias, all_deg_pos)
    if key not in _CACHE:
        _CACHE[key] = build_nc(CL, CH, has_bias, all_deg_pos)
    return _CACHE[key]


def prepare(data, src, tgt, W_pre, b_pre, T_pre, bT_pre,
            W_blk, b_blk, T_blk, bT_blk,
            W_post, b_post, T_post, bT_post):
    """Host preprocessing: returns ((CL, CH, has_bias, all_deg_pos), in_maps)."""
    bf = ml_dtypes.bfloat16
    data = np.asarray(data, np.float32)
    pre = preprocess(src, tgt)
    wb, cb, has_bias = prep_weights(W_pre, T_pre, bT_pre, W_blk, T_blk, bT_blk,
                                    W_post, T_post, bT_post)

    data_pad = np.zeros((NPAD, IN), dtype=bf)
    data_pad[:N] = data.astype(bf)

    common = {"wblob": wb, "cblob": cb}
    in_maps = []
    for c in range(NCORES):
        pc = pre["cores"][c]
        in_maps.append({
            **common,
            "data_sh": data_pad[c * NPC:(c + 1) * NPC].copy(),
            "idx_lo": pc["idx_lo"][:16].copy(),
            "idx_hi": pc["idx_hi"][:16].copy(),
            "tl": pc["tl"],
            "a_own": pc["a_own"].astype(bf),
            "b_own": pc["b_own"],
        })
    return (pre["CL"], pre["CH"], has_bias, pre["all_deg_pos"]), in_maps


def kernel(data, src, tgt, W_pre, b_pre, T_pre, bT_pre,
           W_blk, b_blk, T_blk, bT_blk,
           W_post, b_post, T_post, bT_post,
           _trace=False):
    key, in_maps = prepare(data, src, tgt, W_pre, b_pre, T_pre, bT_pre,
                           W_blk, b_blk, T_blk, bT_blk,
                           W_post, b_post, T_post, bT_post)
    nc = _get_nc(*key)
    res = run_bass_kernel_spmd(nc, in_maps, core_ids=list(range(NCORES)),
                               trace=_trace)
    out = np.concatenate([res.results[c]["out"] for c in range(NCORES)], axis=0)
    out = out[:N, :, None].astype(np.float32)
    if _trace:
        kernel._last_exec_time_ns = res.exec_time_ns
        kernel._last_results = res
    return out

